# revision 1
# baseline (speedup 1.0000x reference)
"""NetVLAD pooling kernel for Trainium2, data-parallel over batch across 8 cores.

Computation per batch b (reference semantics):
  y      = x @ W_red.T + b_red            # [m, 64]
  yn     = y / ||y||_row                  # L2 normalize rows
  logits = yn @ W_lin.T + b_lin           # [m, 8]
  a      = softmax(logits, axis=1)
  vlad   = a.T @ yn - centroids * a.sum(0)[:, None]
  out    = l2norm_global(l2norm_rows(vlad).flatten())

Device-side algebra (per row m):
  yz   = x @ [W_red.T | W_red.T W_lin.T] + [b_red | W_lin b_red]   # fused [m, 72]
  inv  = exp(-0.5 ln(sum(y^2)))        # 1/||y|| via the ln/exp table set
  n    = exp(+0.5 ln(sum(y^2)))        # ||y||
  e    = exp(raw2 * inv)               # un-biased softmax numerator
  r    = 1 / sum_k(e * exp(b_lin))
  atil = e * (inv * r)                 # so atil.T @ [y | n] = [a.T yn | a.sum]
  vlad accumulated in PSUM; row k scaled by exp(b_lin)[k] at finalize.

x is shipped to the device pre-transposed to [b, C, m] in fp8e4m3 so the
contraction dim (C) lands on SBUF partitions with contiguous DMA descriptors,
halving HBM traffic vs bf16 and enabling 4x fast-weight-load on the PE.  The
output is centroid-dominated (||a.T yn|| ~ 21 vs ||centroids * asum|| ~ 4700),
so quantization error lands ~2.8e-4 relative to output scale.
"""
import numpy as np
import ml_dtypes
from contextlib import ExitStack

import concourse.bass as bass
import concourse.tile as tile
import concourse.bass_isa as bass_isa
from concourse import bacc, mybir
from concourse._compat import with_exitstack
from concourse.bass_utils import run_bass_kernel_spmd

bf16 = ml_dtypes.bfloat16
F32 = mybir.dt.float32
BF16 = mybir.dt.bfloat16
FP8 = mybir.dt.float8e4
fp8 = ml_dtypes.float8_e4m3

# tuning toggles (read at program-build time)
XT_FP8 = True        # ship x as fp8e4m3 (else bf16)
W_FP8 = True         # ship Wcat/bias as fp8e4m3 (else bf16); fp8 stationary gets 4x FWL
SQ_DVE = False       # compute y^2 on DVE instead of ACT
AM_POOL = False      # compute am = e*ebl on GpSimd
ATIL_POOL = False    # compute atil = e*q on GpSimd
SB_BUFS = 4
XT_BUFS = 4
YCOPY_ACT_SPLIT = True   # alternate agg-rhs y-copy between DVE and ACT
YCOPY_MOD = 4        # 1 of every MOD y-copies goes to DVE

N_CORES = 8
B, M, C = 32, 8192, 512
K, D = 8, 64
B_LOC = B // N_CORES          # 4 batches per core
M_TILE = 1024
N_TILES = M // M_TILE         # 8
SUB = M_TILE // 128           # 8 subtiles of 128 rows
NCH = C // 128                # 4 contraction chunks


@with_exitstack
def _netvlad_kernel(ctx: ExitStack, tc: tile.TileContext, out_d, xt_d, wcat_d,
                    bcat_d, eblbc_d, ebl8_d, cent_d):
    nc = tc.nc
    AF = mybir.ActivationFunctionType
    OP = mybir.AluOpType

    consts = ctx.enter_context(tc.tile_pool(name="consts", bufs=1))
    xt_pool = ctx.enter_context(tc.tile_pool(name="xt", bufs=XT_BUFS))
    sb = ctx.enter_context(tc.tile_pool(name="work", bufs=SB_BUFS))
    outp = ctx.enter_context(tc.tile_pool(name="outp", bufs=1))
    yz_pool = ctx.enter_context(tc.tile_pool(name="yz", bufs=3, space="PSUM"))
    vlad_pool = ctx.enter_context(tc.tile_pool(name="vlad", bufs=2, space="PSUM"))

    # constants, loaded once
    wcat = consts.tile([128, NCH, 72], FP8 if W_FP8 else BF16)
    nc.sync.dma_start(wcat[:], wcat_d.rearrange("j p t -> p j t"))
    bcat = consts.tile([1, 72], FP8 if W_FP8 else BF16)
    nc.gpsimd.dma_start(bcat[:], bcat_d[:])
    eblbc = consts.tile([128, SUB, K], F32)
    nc.gpsimd.dma_start(eblbc[:], eblbc_d[:])
    ebl8 = consts.tile([K, 1], F32)
    nc.gpsimd.dma_start(ebl8[:], ebl8_d[:])
    cent = consts.tile([K, D], F32)
    nc.gpsimd.dma_start(cent[:], cent_d[:])
    ones = consts.tile([1, 128], FP8 if W_FP8 else BF16)
    nc.vector.memset(ones[:], 1.0)

    outsb = outp.tile([K, B_LOC, D], F32)

    tile_idx = [0]

    def process_tile(vlad, xt_b, m0, rows, first, last):
        sub = rows // 128
        idx = tile_idx[0]
        tile_idx[0] += 1
        xt = xt_pool.tile([128, NCH, rows], FP8 if XT_FP8 else BF16, tag="xt")
        nc.sync.dma_start(xt[:], xt_b[:, :, m0:m0 + rows])

        # fused reduction+logits matmul: yz[m, :72] = x @ Wcat + bcat
        yz = yz_pool.tile([128, sub, 128], F32, tag="yz")
        for s in range(sub):
            for j in range(NCH):
                nc.tensor.matmul(
                    yz[:, s, :72],
                    xt[:, j, s * 128:(s + 1) * 128],
                    wcat[:, j, :],
                    start=(j == 0), stop=False,
                )
            nc.tensor.matmul(yz[:, s, :72], ones[:], bcat[:],
                             start=False, stop=True)

        # ss = sum(y^2) per row; inv = 1/||y|| via ln/exp
        sqs = sb.tile([128, sub, D], BF16, tag="sqs")
        if SQ_DVE:
            nc.vector.tensor_tensor(out=sqs[:], in0=yz[:, :, :D],
                                    in1=yz[:, :, :D], op=OP.mult)
        else:
            nc.scalar.activation(sqs[:], yz[:, :, :D], AF.Square)
        ss8 = sb.tile([128, sub], F32, tag="ss8")
        nc.vector.reduce_sum(ss8[:], sqs[:], axis=mybir.AxisListType.X)
        lss = sb.tile([128, sub], F32, tag="lss")
        nc.scalar.activation(lss[:], ss8[:], AF.Ln)
        inv8 = sb.tile([128, sub], F32, tag="inv8")
        nc.scalar.activation(inv8[:], lss[:], AF.Exp, scale=-0.5)

        # agg rhs = [y | n] in bf16; n = ss * inv (cheaper than exp(+.5 ln))
        rhs = sb.tile([128, sub, D + 1], BF16, tag="rhs")
        if not YCOPY_ACT_SPLIT or idx % YCOPY_MOD == 0:
            nc.vector.tensor_copy(rhs[:, :, :D], yz[:, :, :D])
        else:
            nc.scalar.activation(rhs[:, :, :D], yz[:, :, :D], AF.Copy)
        nc.vector.tensor_tensor(out=rhs[:, :, D:D + 1],
                                in0=ss8[:].unsqueeze(2),
                                in1=inv8[:].unsqueeze(2), op=OP.mult)

        # softmax numerators: e = exp(raw2 * inv)
        t64 = sb.tile([128, sub, K], F32, tag="t64")
        nc.vector.tensor_tensor(
            out=t64[:], in0=yz[:, :, D:D + K],
            in1=inv8[:].unsqueeze(2).broadcast_to([128, sub, K]),
            op=OP.mult)
        e64 = sb.tile([128, sub, K], F32, tag="e64")
        nc.scalar.activation(e64[:], t64[:], AF.Exp)
        # r = 1/sum_k(e * exp(b_lin)); q = inv * r
        am = sb.tile([128, sub, K], F32, tag="am")
        eng_am = nc.gpsimd if AM_POOL else nc.vector
        eng_am.tensor_tensor(out=am[:], in0=e64[:], in1=eblbc[:, :sub, :],
                             op=OP.mult)
        rs8 = sb.tile([128, sub], F32, tag="rs8")
        nc.vector.reduce_sum(rs8[:], am[:], axis=mybir.AxisListType.X)
        rr8 = sb.tile([128, sub], F32, tag="rr8")
        nc.vector.reciprocal(rr8[:], rs8[:])
        q8 = sb.tile([128, sub], F32, tag="q8")
        nc.vector.tensor_tensor(out=q8[:], in0=inv8[:], in1=rr8[:], op=OP.mult)
        atil = sb.tile([128, sub, K], BF16, tag="atil")
        eng_at = nc.gpsimd if ATIL_POOL else nc.vector
        eng_at.tensor_tensor(
            out=atil[:], in0=e64[:],
            in1=q8[:].unsqueeze(2).broadcast_to([128, sub, K]),
            op=OP.mult)

        # vlad[k, :] += atil_s.T @ [y | n]
        for s in range(sub):
            nc.tensor.matmul(
                vlad[:], atil[:, s, :], rhs[:, s, :],
                start=(first and s == 0),
                stop=(last and s == sub - 1),
            )

    for b in range(B_LOC):
        vlad = vlad_pool.tile([K, D + 1], F32, tag="vlad")
        xt_b = xt_d[b].rearrange("(j p) m -> p j m", p=128)
        # halve the very first tile so the DVE/ACT pipeline fills earlier
        spans = ([(0, M_TILE // 2), (M_TILE // 2, M_TILE // 2)]
                 if b == 0 else [(0, M_TILE)])
        spans += [(t * M_TILE, M_TILE) for t in range(1, N_TILES)]
        for i, (m0, rows) in enumerate(spans):
            process_tile(vlad, xt_b, m0, rows,
                         first=(i == 0), last=(i == len(spans) - 1))

        # finalize batch: vlad rows scaled by exp(b_lin), centroid subtract,
        # intra-normalize, global normalize
        vl = sb.tile([K, D + 1], F32)
        nc.vector.tensor_scalar_mul(vl[:], vlad[:], ebl8[:])
        cv = sb.tile([K, D], F32)
        nc.vector.tensor_scalar_mul(cv[:], cent[:], vl[:, D:D + 1])
        v = sb.tile([K, D], F32)
        nc.vector.tensor_sub(v[:], vl[:, :D], cv[:])
        sck = sb.tile([K, D], F32)
        nc.vector.tensor_tensor(out=sck[:], in0=v[:], in1=v[:], op=OP.mult)
        ssk = sb.tile([K, 1], F32)
        nc.vector.reduce_sum(ssk[:], sck[:], axis=mybir.AxisListType.X)
        lk = sb.tile([K, 1], F32)
        nc.scalar.activation(lk[:], ssk[:], AF.Ln)
        invk = sb.tile([K, 1], F32)
        nc.scalar.activation(invk[:], lk[:], AF.Exp, scale=-0.5)
        # after intra-normalization each of the K rows has norm exactly 1,
        # so the global norm is sqrt(K) (fp error ~1e-7, far under budget);
        # fold 1/sqrt(K) into the intra-norm multiply
        nc.vector.tensor_scalar(
            out=outsb[:, b, :], in0=v[:], scalar1=invk[:],
            scalar2=float(1.0 / np.sqrt(K)), op0=OP.mult, op1=OP.mult)

    nc.sync.dma_start(out_d.rearrange("b (k d) -> k b d", k=K), outsb[:])


_CACHE = {}


def _patch_act_tables():
    """Force all Exp/Ln/Square activations to resolve in the one table set
    that contains them all (natural_log_exp_and_others), so bacc's
    insert_act_table_loads emits a single hoisted LoadActFuncSet instead of
    thrashing between exp_and_others and natural_log per tile (~2.7us per
    reload).  List order/length is preserved so act_func_set_id stays a
    valid index into act_info.json."""
    import concourse.bacc as bacc_mod
    import concourse.hw_specs as hw_specs
    if _CACHE.get("act_patched"):
        return
    orig = hw_specs.get_activation_tables
    AF = mybir.ActivationFunctionType
    strip = {AF.Exp, AF.Ln, AF.Square}
    keep = "natural_log_exp_and_others"

    def patched(arch):
        tables = orig(arch)
        return {
            name: (set(fns) if name == keep else set(fns) - strip)
            for name, fns in tables.items()
        }

    bacc_mod.get_activation_tables = patched
    _CACHE["act_patched"] = True


def _declare_io(nc):
    xt_d = nc.dram_tensor("xt", [B_LOC, C, M], FP8 if XT_FP8 else BF16,
                          kind="ExternalInput").ap()
    wcat_d = nc.dram_tensor("wcat", [NCH, 128, 72], FP8 if W_FP8 else BF16,
                            kind="ExternalInput").ap()
    bcat_d = nc.dram_tensor("bcat", [1, 72], FP8 if W_FP8 else BF16,
                            kind="ExternalInput").ap()
    eblbc_d = nc.dram_tensor("eblbc", [128, SUB, K], F32, kind="ExternalInput").ap()
    ebl8_d = nc.dram_tensor("ebl8", [K, 1], F32, kind="ExternalInput").ap()
    cent_d = nc.dram_tensor("cent", [K, D], F32, kind="ExternalInput").ap()
    out_d = nc.dram_tensor("out", [B_LOC, K * D], F32, kind="ExternalOutput").ap()
    return out_d, xt_d, wcat_d, bcat_d, eblbc_d, ebl8_d, cent_d


def _build_program():
    if "nc" in _CACHE:
        return _CACHE["nc"]
    _patch_act_tables()
    nc = bacc.Bacc("TRN2", target_bir_lowering=False, debug=False,
                   num_devices=N_CORES)
    out_d, xt_d, wcat_d, bcat_d, eblbc_d, ebl8_d, cent_d = _declare_io(nc)

    with tile.TileContext(nc) as tc:
        _netvlad_kernel(tc, out_d, xt_d, wcat_d, bcat_d, eblbc_d, ebl8_d, cent_d)
    nc.compile()
    _CACHE["nc"] = nc
    return nc


def _prep_inputs(x, W_red, b_red, W_lin, b_lin, centroids):
    wcat = np.concatenate([W_red.T, W_red.T @ W_lin.T], axis=1)     # [512, 72]
    wcat = np.ascontiguousarray(
        wcat.astype(fp8 if W_FP8 else bf16).reshape(NCH, 128, 72))
    bcat = np.concatenate([b_red, W_lin @ b_red]).astype(
        fp8 if W_FP8 else bf16)[None, :]
    ebl = np.exp(b_lin).astype(np.float32)
    eblbc = np.ascontiguousarray(
        np.broadcast_to(ebl, (128, SUB, K)).astype(np.float32))
    ebl8 = ebl[:, None]
    cent = centroids.astype(np.float32)
    xt = np.ascontiguousarray(
        x.astype(fp8 if XT_FP8 else bf16).transpose(0, 2, 1))      # [B, C, M]
    return xt, wcat, bcat, eblbc, ebl8, cent


def kernel(x, mask, W_red, b_red, W_lin, b_lin, centroids, **kwargs):
    x = np.asarray(x, dtype=np.float32)
    W_red = np.asarray(W_red, dtype=np.float32)
    b_red = np.asarray(b_red, dtype=np.float32)
    W_lin = np.asarray(W_lin, dtype=np.float32)
    b_lin = np.asarray(b_lin, dtype=np.float32)
    centroids = np.asarray(centroids, dtype=np.float32)

    xt, wcat, bcat, eblbc, ebl8, cent = _prep_inputs(
        x, W_red, b_red, W_lin, b_lin, centroids)

    nc = _build_program()
    in_maps = []
    for i in range(N_CORES):
        in_maps.append({
            "xt": np.ascontiguousarray(xt[i * B_LOC:(i + 1) * B_LOC]),
            "wcat": wcat, "bcat": bcat, "eblbc": eblbc,
            "ebl8": ebl8, "cent": cent,
        })
    res = run_bass_kernel_spmd(nc, in_maps, list(range(N_CORES)),
                               **kwargs.get("_run_kwargs", {}))
    out = np.concatenate([res.results[i]["out"] for i in range(N_CORES)], axis=0)
    if kwargs.get("_return_raw"):
        return out, res
    return out

